# revision 1
# baseline (speedup 1.0000x reference)
"""Trainium2 Bass kernel v2 for nn_DynamicGraphAttention (kNN EdgeConv + max-pool).

Same math as the validated baseline (see kernel.py docstring), restructured:
  - The full score s(n,m) = fl(fl(2dot - pn^2) - pm^2) is accumulated in PSUM:
      matmul fp32 K=3 (2*pn x pm)          -> 2dot (bitwise 2x the raw dot)
      matmul bf16 K=3 (pn^2 parts x ones)  -> single-rounded += -pn^2
      matmul bf16 K=3 (ones x pm^2 parts)  -> single-rounded += -pm^2
    (3-term bf16 splits are exact and every partial sum is exactly
    representable, so each fold is one fp32-rounded PSUM add -- verified
    bitwise vs the ACT+DVE chain on HW.)  max8/max_index read scores straight
    from PSUM: the ACT identity pass and the DVE -pm^2 broadcast add are gone.
  - Position operands (2*pnT, pmT, bf16 norm splits) are prepared on CPU.
  - A = q@W0 and Bp = qr@(W1-W0)+b are interleaved into chunk 0's row tiles.
  - Stage-2 u16->f32 casts run on the Scalar engine; gather stays SWDGE.

Sharding: 8 cores = (4 batches) x (2 row-halves of 4096), as baseline.
"""

import numpy as np
import ml_dtypes

import concourse.bacc as bacc
import concourse.bass as bass
import concourse.mybir as mybir
from concourse.bass_utils import run_bass_kernel_spmd
from concourse.masks import make_identity
from concourse.tile import TileContext

F32 = mybir.dt.float32
BF16 = mybir.dt.bfloat16
I16 = mybir.dt.int16
U16 = mybir.dt.uint16
U32 = mybir.dt.uint32

P = 128
N = 8192
NR = 4096
C = 256
GS = 512
G = N // GS          # 16 groups
NT = NR // P         # 32 row tiles per core
QT = N // P          # 64 A tiles per batch
K = 16
NEG_BIG = -3.0e38
CHUNK_TILES = 8
NCHUNK = NT // CHUNK_TILES   # 4 chunks of 1024 rows
import os
ADD_DVE_X = int(os.environ.get("ADD_DVE_X", "6"))  # of 10 groups, ADD on DVE


def build_nc():
    nc = bacc.Bacc("TRN2", target_bir_lowering=False)
    AL = mybir.AluOpType
    AF = mybir.ActivationFunctionType

    q_d = nc.dram_tensor("q", [N, C], F32, kind="ExternalInput")
    qr_d = nc.dram_tensor("qr", [NR, C], F32, kind="ExternalInput")
    ls2_d = nc.dram_tensor("ls2", [3, NR], F32, kind="ExternalInput")
    rs_d = nc.dram_tensor("rs", [3, N], F32, kind="ExternalInput")
    pn2s_d = nc.dram_tensor("pn2s", [P, NT], F32, kind="ExternalInput")
    pm2n_d = nc.dram_tensor("pm2n", [1, N], F32, kind="ExternalInput")
    w_d = nc.dram_tensor("w", [2 * C, C], F32, kind="ExternalInput")
    b_d = nc.dram_tensor("bvec", [1, C], F32, kind="ExternalInput")
    out_d = nc.dram_tensor("out", [NR, C], F32, kind="ExternalOutput")
    idx_d = nc.dram_tensor("idx_out", [NR, K], U32, kind="ExternalOutput")

    a_d = nc.dram_tensor("a_scratch", [N, C], F32)
    bp_d = nc.dram_tensor("bp_scratch", [NR, C], F32)
    widx_d = nc.dram_tensor("widx_scratch", [NCHUNK, K, 2, CHUNK_TILES, 64], I16)

    with TileContext(nc) as tc:
        with (
            tc.tile_pool(name="const", bufs=1) as const,
            tc.tile_pool(name="small", bufs=5) as small,
        ):
            identity = const.tile([P, P], F32, tag="identity")
            make_identity(nc, identity)

            w_sb = const.tile([P, 4, C], F32, tag="w_sb")
            wd_sb = const.tile([P, 2, C], F32, tag="wd_sb")
            bias_sb = const.tile([1, C], F32, tag="bias_sb")
            ones1 = const.tile([1, P], F32, tag="ones1")
            rs_sb = const.tile([3, N], F32, tag="rs_sb")
            ls2_sb = const.tile([3, NR], F32, tag="ls2_sb")
            negpn2 = const.tile([P, NT], F32, tag="negpn2")
            negpm2b = const.tile([P, N], F32, tag="negpm2b")
            goffs_f = const.tile([P, G, 8], F32, tag="goffs_f")
            iota_f = const.tile([P, P], F32, tag="iota_f")

            nc.sync.dma_start(
                out=w_sb, in_=w_d[:].rearrange("(ch p) c -> p ch c", p=P)
            )
            nc.sync.dma_start(out=bias_sb, in_=b_d[:])
            nc.sync.dma_start(out=rs_sb, in_=rs_d[:])
            nc.sync.dma_start(out=ls2_sb, in_=ls2_d[:])
            nc.sync.dma_start(out=negpn2, in_=pn2s_d[:])
            nc.gpsimd.dma_start(
                out=negpm2b,
                in_=bass.AP(pm2n_d[0].tensor, pm2n_d[0].offset, [[0, P], [1, N]]),
            )
            nc.vector.memset(ones1, 1.0)
            nc.vector.tensor_sub(wd_sb, w_sb[:, 2:4], w_sb[:, 0:2])

            goffs_i = small.tile([P, G, 8], mybir.dt.int32, tag="goffs_i")
            nc.gpsimd.iota(goffs_i, pattern=[[GS, G], [0, 8]], channel_multiplier=0)
            nc.vector.tensor_copy(goffs_f, goffs_i)
            iota_i = small.tile([P, P], mybir.dt.int32, tag="iota_i")
            nc.gpsimd.iota(iota_i, pattern=[[1, P]], channel_multiplier=0)
            nc.vector.tensor_copy(iota_f, iota_i)

            # strided views: row-tile (cc, w) covers rows
            # n = 1024*cc + 16*lo + 8*hi + w with partition p = 64*hi + lo
            # ls2 is CPU-prepermuted to [k, cc, w, p] order -> contiguous slices
            ls2_v = ls2_sb.rearrange("k (c w p) -> k c w p", w=CHUNK_TILES, p=P)
            out_tiled = out_d[:].rearrange("(t p) c -> p t c", p=P)
            idx_strided = idx_d[:].rearrange("(c l h w) k -> c w h l k", l=64, h=2, w=8)
            bp_tiled = bp_d[:].rearrange("(t p) c -> p t c", p=P)
            q_tiled = q_d[:].rearrange("(t p) c -> t p c", p=P)
            qr_tiled = qr_d[:].rearrange("(t p) c -> t p c", p=P)
            a_tiled = a_d[:].rearrange("(t p) c -> t p c", p=P)
            bp_w = bp_d[:].rearrange("(t p) c -> t p c", p=P)

            with (
                tc.tile_pool(name="spsum", bufs=3, space="PSUM") as spsum,
                tc.tile_pool(name="apsum", bufs=2, space="PSUM") as apsum,
                tc.tile_pool(name="gtpsum", bufs=1, space="PSUM") as gtpsum,
                tc.tile_pool(name="awork", bufs=3) as awork,
                tc.tile_pool(name="swork", bufs=4) as swork,
                tc.tile_pool(name="mwork", bufs=4) as mwork,
            ):
                gstate = {}
                NIDX = CHUNK_TILES * P  # 1024 = SWDGE cap, one call per j

                def gather_begin(cc):
                    idxs_all = mwork.tile([P, K, 64], I16, tag="idxs_all")
                    for cb in range(8):
                        nc.sync.dma_start(
                            out=idxs_all[16 * cb : 16 * (cb + 1)],
                            in_=widx_d[cc].rearrange("j uh w m -> (uh w) j m"),
                        )
                    gacc = mwork.tile([P, CHUNK_TILES, C], F32, tag="gacc")
                    gstate[cc] = {"idxs": idxs_all, "gacc": gacc, "dsts": {}}

                def gather_issue(cc, j):
                    st = gstate[cc]
                    dst = (
                        st["gacc"]
                        if j == 0
                        else mwork.tile([P, CHUNK_TILES, C], F32, tag="gtmp")
                    )
                    nc.gpsimd.dma_gather(
                        out_ap=dst,
                        in_ap=a_d[:],
                        idxs_ap=st["idxs"][:, j],
                        num_idxs=NIDX,
                        num_idxs_reg=NIDX,
                        elem_size=C,
                    )
                    st["dsts"][j] = dst

                def gather_max(cc, j):
                    st = gstate[cc]
                    if j < 1:
                        return
                    dst = st["dsts"].pop(j)
                    nc.vector.tensor_tensor(
                        out=st["gacc"], in0=st["gacc"], in1=dst, op=AL.max
                    )

                def gather_finish(cc):
                    st = gstate.pop(cc)
                    gacc = st["gacc"]
                    t0 = cc * CHUNK_TILES
                    bpt = mwork.tile([P, CHUNK_TILES, C], F32, tag="gtmp")
                    nc.sync.dma_start(out=bpt, in_=bp_tiled[:, t0 : t0 + CHUNK_TILES])
                    nc.vector.tensor_add(gacc, gacc, bpt)
                    osb = mwork.tile([P, CHUNK_TILES, C], F32, tag="gtmp")
                    nc.vector.scalar_tensor_tensor(
                        out=osb, in0=gacc, scalar=0.2, in1=gacc,
                        op0=AL.mult, op1=AL.max,
                    )
                    nc.sync.dma_start(
                        out=out_tiled[:, t0 : t0 + CHUNK_TILES], in_=osb
                    )

                for cc in range(NCHUNK):
                    for w in range(CHUNK_TILES):
                        # ---- interleaved A/Bp tiles (front-loaded in chunk 0)
                        # per chunk-0 row-tile: 4 A tiles + 2 Bp tiles -> all
                        # 64 A and 32 Bp tiles are done when chunk-0 gathers
                        # start.
                        if cc == 0:
                            for sub in range(8):
                                t = w * 8 + sub
                                qtile = awork.tile([P, C], F32, tag="qtile")
                                nc.sync.dma_start(out=qtile, in_=q_tiled[t])
                                qt_ps = apsum.tile([P, 2, P], F32, tag="qt_ps")
                                for ch in range(2):
                                    nc.tensor.transpose(
                                        qt_ps[:, ch],
                                        qtile[:, ch * P : (ch + 1) * P],
                                        identity,
                                    )
                                qt_sb = awork.tile([P, 2, P], F32, tag="qt_sb")
                                nc.scalar.copy(qt_sb, qt_ps)
                                a_ps = apsum.tile([P, C], F32, tag="a_ps")
                                for ch in range(2):
                                    nc.tensor.matmul(
                                        a_ps,
                                        qt_sb[:, ch],
                                        w_sb[:, ch],
                                        start=(ch == 0),
                                        stop=(ch == 1),
                                    )
                                a_sb = awork.tile([P, C], F32, tag="a_sb")
                                nc.scalar.copy(a_sb, a_ps)
                                nc.sync.dma_start(out=a_tiled[t], in_=a_sb)
                            for sub in range(4):
                                t = w * 4 + sub
                                qtile = awork.tile([P, C], F32, tag="qtile")
                                nc.sync.dma_start(out=qtile, in_=qr_tiled[t])
                                qt_ps = apsum.tile([P, 2, P], F32, tag="qt_ps")
                                for ch in range(2):
                                    nc.tensor.transpose(
                                        qt_ps[:, ch],
                                        qtile[:, ch * P : (ch + 1) * P],
                                        identity,
                                    )
                                qt_sb = awork.tile([P, 2, P], F32, tag="qt_sb")
                                nc.scalar.copy(qt_sb, qt_ps)
                                bp_ps = apsum.tile([P, C], F32, tag="a_ps")
                                nc.tensor.matmul(
                                    bp_ps, qt_sb[:, 0], wd_sb[:, 0],
                                    start=True, stop=False,
                                )
                                nc.tensor.matmul(
                                    bp_ps, qt_sb[:, 1], wd_sb[:, 1],
                                    start=False, stop=False,
                                )
                                nc.tensor.matmul(
                                    bp_ps, ones1, bias_sb,
                                    start=False, stop=True,
                                )
                                bp_sb = awork.tile([P, C], F32, tag="a_sb")
                                nc.scalar.copy(bp_sb, bp_ps)
                                nc.sync.dma_start(out=bp_w[t], in_=bp_sb)

                        # ---- scores + per-group top-8, straight from PSUM
                        ts = cc * CHUNK_TILES + w
                        v8 = small.tile([P, G, 8], F32, tag="v8")
                        i8 = small.tile([P, G, 8], U16, tag="i8")
                        lh2 = ls2_v[:, cc, w]
                        for g in range(G):
                            s_ps = spsum.tile([P, GS], F32, tag="s_ps")
                            nc.tensor.matmul(
                                s_ps, lh2, rs_sb[:, g * GS : (g + 1) * GS],
                                start=True, stop=True,
                            )
                            s1 = swork.tile([P, GS], F32, tag="s1")
                            nc.scalar.activation(
                                s1, s_ps, AF.Identity,
                                bias=negpn2[:, ts : ts + 1], scale=1.0,
                            )
                            s_sb = swork.tile([P, GS], F32, tag="s_sb")
                            xx = 4 if cc == 0 else 8
                            eng = nc.vector if (g % 10) < xx else nc.gpsimd
                            eng.tensor_tensor(
                                out=s_sb, in0=s1,
                                in1=negpm2b[:, g * GS : (g + 1) * GS],
                                op=AL.add,
                            )
                            nc.vector.max(out=v8[:, g], in_=s_sb)
                            nc.vector.max_index(
                                out=i8[:, g], in_max=v8[:, g], in_values=s_sb
                            )

                        # ---- stage 2: top-16 of the 128 candidates
                        locf = small.tile([P, G, 8], F32, tag="locf")
                        nc.scalar.copy(locf, i8)  # u16 -> f32 cast on ACT
                        gidxf = small.tile([P, P], F32, tag="gidxf")
                        nc.gpsimd.tensor_tensor(
                            out=gidxf,
                            in0=locf.rearrange("p g s -> p (g s)"),
                            in1=goffs_f.rearrange("p g s -> p (g s)"),
                            op=AL.add,
                        )
                        cand = v8.rearrange("p g s -> p (g s)")
                        t8a = small.tile([P, 8], F32, tag="t8a")
                        t8b = small.tile([P, 8], F32, tag="t8b")
                        p16 = small.tile([P, K], U16, tag="p16")
                        cand2 = small.tile([P, P], F32, tag="cand2")
                        nc.vector.max(out=t8a, in_=cand)
                        nc.vector.max_index(out=p16[:, 0:8], in_max=t8a, in_values=cand)
                        nc.vector.match_replace(
                            out=cand2, in_to_replace=t8a, in_values=cand,
                            imm_value=NEG_BIG,
                        )
                        nc.vector.max(out=t8b, in_=cand2)
                        nc.vector.max_index(
                            out=p16[:, 8:16], in_max=t8b, in_values=cand2
                        )
                        p16f = small.tile([P, K], F32, tag="p16f")
                        nc.scalar.copy(p16f, p16)  # u16 -> f32 cast on ACT

                        gself = small.tile([P, K], F32, tag="gself")
                        sttscr = small.tile([P, P], F32, tag="sttscr")
                        for j in range(K):
                            nc.vector.scalar_tensor_tensor(
                                out=sttscr,
                                in0=iota_f,
                                scalar=p16f[:, j : j + 1],
                                in1=gidxf,
                                op0=AL.is_equal,
                                op1=AL.mult,
                                accum_out=gself[:, j : j + 1],
                            )
                        gt_ps = gtpsum.tile([16, P], F32, tag="gt_ps")
                        nc.tensor.transpose(gt_ps, gself, identity)
                        gtw = small.tile([16, P], I16, tag="gtw")
                        nc.vector.tensor_copy(gtw, gt_ps)
                        nc.sync.dma_start(
                            out=widx_d[cc, :, :, w],
                            in_=gtw.rearrange("j (h m) -> j h m", h=2),
                        )

                        idxu = small.tile([P, K], U32, tag="idxu")
                        nc.vector.tensor_copy(idxu, gself)
                        nc.sync.dma_start(out=idx_strided[cc, w], in_=idxu)

                        if cc >= 1:
                            if w == 0:
                                gather_begin(cc - 1)
                            gather_issue(cc - 1, 2 * w)
                            gather_issue(cc - 1, 2 * w + 1)
                            if w >= 1:
                                gather_max(cc - 1, 2 * w - 2)
                                gather_max(cc - 1, 2 * w - 1)
                    if cc >= 1:
                        gather_max(cc - 1, 14)
                        gather_max(cc - 1, 15)
                        gather_finish(cc - 1)
                lastc = NCHUNK - 1
                gather_begin(lastc)
                for j in range(K):
                    gather_issue(lastc, j)
                    gather_max(lastc, j - 1)
                gather_max(lastc, K - 1)
                gather_finish(lastc)

    nc.compile()
    return nc


_NC_CACHE = None


def _get_nc():
    global _NC_CACHE
    if _NC_CACHE is None:
        _NC_CACHE = build_nc()
    return _NC_CACHE


def _split3(v):
    v = np.asarray(v, dtype=np.float32)
    h = v.astype(ml_dtypes.bfloat16)
    r = (v - h.astype(np.float32)).astype(np.float32)
    m = r.astype(ml_dtypes.bfloat16)
    r2 = (r - m.astype(np.float32)).astype(np.float32)
    l = r2.astype(ml_dtypes.bfloat16)
    assert np.all(r2 - l.astype(np.float32) == 0)
    return np.stack([h, m, l])


def _shard_inputs(q, q_pos, W, b):
    q = np.ascontiguousarray(np.asarray(q, dtype=np.float32))
    q_pos = np.ascontiguousarray(np.asarray(q_pos, dtype=np.float32))
    W = np.ascontiguousarray(np.asarray(W, dtype=np.float32))
    b = np.ascontiguousarray(np.asarray(b, dtype=np.float32)).reshape(1, C)
    in_maps = []
    for core in range(8):
        bi, h = divmod(core, 2)
        rows = slice(h * NR, (h + 1) * NR)
        posb = q_pos[bi]                      # [8192, 3]
        posr = np.ascontiguousarray(posb[rows])   # [4096, 3]
        pn2 = (posr * posr).sum(-1, dtype=np.float32)
        pm2 = (posb * posb).sum(-1, dtype=np.float32)
        # negpn2 strided: [p=(hi,lo), ts=(cc,w)] = -pn2[1024cc + 16lo + 8hi + w]
        npn = (-pn2).reshape(NCHUNK, 64, 2, CHUNK_TILES)  # [cc, lo, hi, w]
        pn2s = np.ascontiguousarray(npn.transpose(2, 1, 0, 3).reshape(P, NT))
        in_maps.append(
            {
                "q": q[bi],
                "qr": np.ascontiguousarray(q[bi, rows]),
                "ls2": np.ascontiguousarray(
                    (posr * np.float32(2.0)).T
                    .reshape(3, NCHUNK, 64, 2, CHUNK_TILES)
                    .transpose(0, 1, 4, 3, 2)
                    .reshape(3, NR)
                ),
                "rs": np.ascontiguousarray(posb.T),
                "pn2s": pn2s,
                "pm2n": np.ascontiguousarray((-pm2).reshape(1, N)),
                "w": W,
                "bvec": b,
            }
        )
    return in_maps


def run_on_hw(q, q_pos, W, b, trace=False):
    """Run the SPMD kernel on the 8 cores; returns (out[4,8192,256], results)."""
    nc = _get_nc()
    in_maps = _shard_inputs(q, q_pos, W, b)
    res = run_bass_kernel_spmd(nc, in_maps, core_ids=list(range(8)), trace=trace)
    out = np.empty((4, N, C), dtype=np.float32)
    for core in range(8):
        bi, h = divmod(core, 2)
        out[bi, h * NR : (h + 1) * NR] = res.results[core]["out"]
    return out, res


def kernel(q, q_pos, W, b, k):
    assert int(k) == K, f"kernel hardcodes k=16, got {k}"
    out, _ = run_on_hw(q, q_pos, W, b)
    return out



# revision 13
# speedup vs baseline: 1.2283x; 1.2283x over previous
"""Trainium2 Bass kernel v9 for nn_DynamicGraphAttention (kNN EdgeConv + max-pool).

Exact-selection structure (bitwise-verified chain, global column order so
score ties resolve to the lowest global index exactly like jax top_k):
    s_ps = matmul fp32 K=3 (2pn x pm)      [2dot == jax einsum bitwise]
    s    = DVE scalar_tensor_tensor:       [fl(fl(2dot - pn^2) - pm^2),
           (s_ps + (-pn^2)) + (-pm^2)       two fp32 roundings in ref order]
    per-group top-8 via DVE max8/max_index, two-stage top-16.

vs the 2.05ms v2 baseline:
  - The ACT identity pass and separate DVE/GpSimd -pm^2 adds are ONE
    DVE STT op per 2 groups, straight from a 2-bank PSUM tile: no cross-
    engine hops between matmul and selection.
  - q supplied pre-transposed per core (qT full + qrT own half): no PE
    transposes; A tiles interleave with chunk-0/1 score sets.
  - Row chunks [8,8,8,4,2,2] (mapping n = off + i, i = ch + 16*s,
    ch = w*rr + p//SC, s = p%SC): the final 2-tile chunk shrinks the
    un-overlapped gather tail; chunk-0 gathers run in chunk-1's second half
    so the A phase can spill into chunk-1's first half.
  - Gather k-way max folds split DVE/GpSimd; Bp add rides a SWDGE accum DMA.
  - idx_out debug output (tile-order rows) only when BASS_IDX_OUT=1.

Sharding: 8 cores = (4 batches) x (2 row-halves of 4096), as baseline.
"""

import os

import numpy as np

import concourse.bacc as bacc
import concourse.bass as bass
import concourse.mybir as mybir
from concourse.bass_utils import run_bass_kernel_spmd
from concourse.masks import make_identity
from concourse.tile import TileContext

F32 = mybir.dt.float32
I16 = mybir.dt.int16
U16 = mybir.dt.uint16
U32 = mybir.dt.uint32

P = 128
N = 8192
NR = 4096
C = 256
GS = 512
G = N // GS          # 16 groups of columns
NT = NR // P         # 32 row tiles per core
K = 16
NEG_BIG = -3.0e38
CHUNKS = [8, 8, 8, 4, 2, 2]          # tiles per chunk (each divides 16)
NCH = len(CHUNKS)
COFF = [0]
for _ct in CHUNKS:
    COFF.append(COFF[-1] + _ct)
SOFF = [8 * o for o in COFF]         # widx s-offsets per chunk
SB = 2                               # score groups per PSUM tile / STT op

IDX_OUT = os.environ.get("BASS_IDX_OUT", "0") == "1"

PERMUTED_IDX = False


def _gather_plan():
    """issue_at/fold_at[(ci,w)] lists of (src_chunk, j); END_OPS[ci] =
    (src, trailing js) folded+finished after chunk ci's tile loop."""
    issue_at = {}
    fold_at = {}
    end_ops = {}
    for src in range(NCH - 1):
        host = src + 1
        ct_h = CHUNKS[host]
        if src == 0:
            host_tiles = list(range(ct_h // 2, ct_h))   # second half of c1
        else:
            host_tiles = list(range(ct_h))
        per = -(-K // len(host_tiles))  # ceil
        assign = {}
        for j in range(K):
            w = host_tiles[min(j // per, len(host_tiles) - 1)]
            assign.setdefault(w, []).append(j)
        prev = []
        for w in host_tiles:
            issue_at.setdefault((host, w), []).extend(
                (src, j) for j in assign.get(w, [])
            )
            if prev:
                fold_at.setdefault((host, w), []).extend((src, j) for j in prev)
            prev = assign.get(w, [])
        end_ops[host] = (src, list(prev))
    return issue_at, fold_at, end_ops


ISSUE_AT, FOLD_AT, END_OPS = _gather_plan()


def build_nc():
    nc = bacc.Bacc("TRN2", target_bir_lowering=False)
    AL = mybir.AluOpType

    qt_d = nc.dram_tensor("qT", [C, N], F32, kind="ExternalInput")
    qrt_d = nc.dram_tensor("qrT", [C, NR], F32, kind="ExternalInput")
    ls2_d = nc.dram_tensor("ls2", [3, NR], F32, kind="ExternalInput")
    rs_d = nc.dram_tensor("rs", [3, N], F32, kind="ExternalInput")
    pn2s_d = nc.dram_tensor("pn2s", [P, NT], F32, kind="ExternalInput")
    pm2n_d = nc.dram_tensor("pm2n", [1, N], F32, kind="ExternalInput")
    w_d = nc.dram_tensor("w", [2 * C, C], F32, kind="ExternalInput")
    b_d = nc.dram_tensor("bvec", [1, C], F32, kind="ExternalInput")
    out_d = nc.dram_tensor("out", [NR, C], F32, kind="ExternalOutput")
    if IDX_OUT:
        idx_d = nc.dram_tensor("idx_out", [NR, K], U32, kind="ExternalOutput")

    a_d = nc.dram_tensor("a_scratch", [N, C], F32)
    bp_d = nc.dram_tensor("bp_scratch", [NR, C], F32)
    widx_d = nc.dram_tensor("widx_scratch", [K, 16, SOFF[NCH]], I16)

    with TileContext(nc) as tc:
        with (
            tc.tile_pool(name="const", bufs=1) as const,
            tc.tile_pool(name="small", bufs=5) as small,
        ):
            identity = const.tile([P, P], F32, tag="identity")
            make_identity(nc, identity)

            w_sb = const.tile([P, 4, C], F32, tag="w_sb")
            wd_sb = const.tile([P, 2, C], F32, tag="wd_sb")
            bias_sb = const.tile([1, C], F32, tag="bias_sb")
            ones1 = const.tile([1, P], F32, tag="ones1")
            rs_sb = const.tile([3, N], F32, tag="rs_sb")
            ls2_sb = const.tile([3, NR], F32, tag="ls2_sb")
            negpn2 = const.tile([P, NT], F32, tag="negpn2")
            negpm2b = const.tile([P, N], F32, tag="negpm2b")
            goffs_f = const.tile([P, G, 8], F32, tag="goffs_f")
            iota_f = const.tile([P, P], F32, tag="iota_f")

            nc.sync.dma_start(
                out=w_sb, in_=w_d[:].rearrange("(ch p) c -> p ch c", p=P)
            )
            nc.sync.dma_start(out=bias_sb, in_=b_d[:])
            nc.sync.dma_start(out=rs_sb, in_=rs_d[:])
            nc.sync.dma_start(out=ls2_sb, in_=ls2_d[:])
            nc.sync.dma_start(out=negpn2, in_=pn2s_d[:])
            nc.gpsimd.dma_start(
                out=negpm2b,
                in_=bass.AP(pm2n_d[0].tensor, pm2n_d[0].offset, [[0, P], [1, N]]),
            )
            nc.vector.memset(ones1, 1.0)
            nc.vector.tensor_sub(wd_sb, w_sb[:, 2:4], w_sb[:, 0:2])

            goffs_i = small.tile([P, G, 8], mybir.dt.int32, tag="goffs_i")
            nc.gpsimd.iota(goffs_i, pattern=[[GS, G], [0, 8]], channel_multiplier=0)
            nc.vector.tensor_copy(goffs_f, goffs_i)
            iota_i = small.tile([P, P], mybir.dt.int32, tag="iota_i")
            nc.gpsimd.iota(iota_i, pattern=[[1, P]], channel_multiplier=0)
            nc.vector.tensor_copy(iota_f, iota_i)

            out_tiled = out_d[:].rearrange("(t p) c -> p t c", p=P)
            bp_tiled = bp_d[:].rearrange("(t p) c -> p t c", p=P)
            qt_tiled = qt_d[:].rearrange("(ch p) (t n) -> t p ch n", p=P, n=P)
            qrt_tiled = qrt_d[:].rearrange("(ch p) (t n) -> t p ch n", p=P, n=P)
            a_tiled = a_d[:].rearrange("(t p) c -> t p c", p=P)
            bp_w = bp_d[:].rearrange("(t p) c -> t p c", p=P)

            with (
                tc.tile_pool(name="spsum", bufs=2, space="PSUM") as spsum,
                tc.tile_pool(name="apsum", bufs=2, space="PSUM") as apsum,
                tc.tile_pool(name="gtpsum", bufs=1, space="PSUM") as gtpsum,
                tc.tile_pool(name="awork", bufs=3) as awork,
                tc.tile_pool(name="swork", bufs=6) as swork,
                tc.tile_pool(name="pidx", bufs=2) as pidx,
                tc.tile_pool(name="pgacc", bufs=2) as pgacc,
                tc.tile_pool(name="pgtmp", bufs=4) as pgtmp,
            ):
                gstate = {}

                def gather_begin(ci):
                    ct = CHUNKS[ci]
                    sw = 8 * ct
                    idxs_all = pidx.tile([P, K, 64], I16, tag="idxs_all")
                    src = widx_d[:, :, SOFF[ci] : SOFF[ci] + sw]
                    for cb in range(8):
                        nc.sync.dma_start(
                            out=idxs_all[16 * cb : 16 * (cb + 1), :, :sw],
                            in_=src.rearrange("j ch s -> ch j s"),
                        )
                    gacc = pgacc.tile([P, CHUNKS[0], C], F32, tag="gacc")
                    gstate[ci] = {"idxs": idxs_all, "gacc": gacc, "dsts": {},
                                  "ct": ct}

                def gather_issue(ci, j):
                    if ci not in gstate:
                        gather_begin(ci)
                    st = gstate[ci]
                    ct = st["ct"]
                    dst = (
                        st["gacc"]
                        if j == 0
                        else pgtmp.tile([P, CHUNKS[0], C], F32, tag="gtmp")
                    )
                    nc.gpsimd.dma_gather(
                        out_ap=dst[:, :ct],
                        in_ap=a_d[:],
                        idxs_ap=st["idxs"][:, j, : 8 * ct],
                        num_idxs=P * ct,
                        num_idxs_reg=P * ct,
                        elem_size=C,
                    )
                    st["dsts"][j] = dst

                def gather_max(ci, j, force_dve=False):
                    st = gstate[ci]
                    if j < 1 or j not in st["dsts"]:
                        return
                    ct = st["ct"]
                    dst = st["dsts"].pop(j)
                    nc.vector.tensor_tensor(
                        out=st["gacc"][:, :ct], in0=st["gacc"][:, :ct],
                        in1=dst[:, :ct], op=AL.max,
                    )

                def gather_finish(ci):
                    st = gstate.pop(ci)
                    ct = st["ct"]
                    gacc = st["gacc"]
                    t0 = COFF[ci]
                    nc.gpsimd.dma_start(
                        out=gacc[:, :ct],
                        in_=bp_tiled[:, t0 : t0 + ct],
                        accum_op=AL.add,
                    )
                    osb = pgtmp.tile([P, CHUNKS[0], C], F32, tag="gtmp")
                    nc.vector.scalar_tensor_tensor(
                        out=osb[:, :ct], in0=gacc[:, :ct], scalar=0.2,
                        in1=gacc[:, :ct], op0=AL.mult, op1=AL.max,
                    )
                    nc.sync.dma_start(
                        out=out_tiled[:, t0 : t0 + ct], in_=osb[:, :ct]
                    )

                def ab_tile(t, bp_t):
                    qt_sb = awork.tile([P, 2, P], F32, tag="qt_sb")
                    nc.scalar.dma_start(out=qt_sb, in_=qt_tiled[t])
                    a_ps = apsum.tile([P, C], F32, tag="a_ps")
                    for ch in range(2):
                        nc.tensor.matmul(
                            a_ps, qt_sb[:, ch], w_sb[:, ch],
                            start=(ch == 0), stop=(ch == 1),
                        )
                    a_sb = awork.tile([P, C], F32, tag="a_sb")
                    nc.scalar.copy(a_sb, a_ps)
                    nc.scalar.dma_start(out=a_tiled[t], in_=a_sb)
                    if bp_t is not None:
                        qb_sb = awork.tile([P, 2, P], F32, tag="qt_sb")
                        nc.scalar.dma_start(out=qb_sb, in_=qrt_tiled[bp_t])
                        bp_ps = apsum.tile([P, C], F32, tag="a_ps")
                        nc.tensor.matmul(
                            bp_ps, qb_sb[:, 0], wd_sb[:, 0],
                            start=True, stop=False,
                        )
                        nc.tensor.matmul(
                            bp_ps, qb_sb[:, 1], wd_sb[:, 1],
                            start=False, stop=False,
                        )
                        nc.tensor.matmul(
                            bp_ps, ones1, bias_sb,
                            start=False, stop=True,
                        )
                        bp_sb = awork.tile([P, C], F32, tag="a_sb")
                        nc.scalar.copy(bp_sb, bp_ps)
                        nc.scalar.dma_start(out=bp_w[bp_t], in_=bp_sb)

                # A/Bp emission plan: 48 tiles in chunk 0 (Bp all within),
                # 16 tiles in chunk 1's first half
                ab_queue = [(t, t if t < 32 else None) for t in range(64)]
                ab_pos = 0

                for ci in range(NCH):
                    ct = CHUNKS[ci]
                    rr = 16 // ct
                    sc = 8 * ct
                    for w in range(ct):
                        ts = COFF[ci] + w
                        lh2 = ls2_sb[:, ts * P : (ts + 1) * P]
                        v8 = small.tile([P, G, 8], F32, tag="v8")
                        i8 = small.tile([P, G, 8], U16, tag="i8")
                        nset = G // SB
                        for sset in range(nset):
                            g0 = sset * SB
                            s_ps = spsum.tile([P, SB, GS], F32, tag="s_ps")
                            for gi in range(SB):
                                g = g0 + gi
                                nc.tensor.matmul(
                                    s_ps[:, gi], lh2,
                                    rs_sb[:, g * GS : (g + 1) * GS],
                                    start=True, stop=True,
                                )
                            s_sb = swork.tile([P, SB, GS], F32, tag="s_sb")
                            # s = (s_ps - pn^2) - pm^2 : exact 2-rounding chain
                            nc.vector.scalar_tensor_tensor(
                                out=s_sb.rearrange("p a b -> p (a b)"),
                                in0=s_ps.rearrange("p a b -> p (a b)"),
                                scalar=negpn2[:, ts : ts + 1],
                                in1=negpm2b[:, g0 * GS : (g0 + SB) * GS],
                                op0=AL.add,
                                op1=AL.add,
                            )
                            for gi in range(SB):
                                g = g0 + gi
                                nc.vector.max(out=v8[:, g], in_=s_sb[:, gi])
                                nc.vector.max_index(
                                    out=i8[:, g], in_max=v8[:, g],
                                    in_values=s_sb[:, gi],
                                )
                            # A/Bp interleave: 6/tile in chunk 0, 4/tile in
                            # chunk 1's first half
                            if ci == 0:
                                want = (ts * nset + sset + 1) * 6 // nset
                            elif ci == 1 and w < ct // 2:
                                want = 48 + (w * nset + sset + 1) * 4 // nset
                            else:
                                want = ab_pos
                            while ab_pos < min(want, 64):
                                ab_tile(*ab_queue[ab_pos])
                                ab_pos += 1

                        # ---- stage 2: top-16 of the 128 candidates (all DVE)
                        locf = small.tile([P, G, 8], F32, tag="locf")
                        nc.vector.tensor_copy(locf, i8)
                        gidxf = small.tile([P, P], F32, tag="gidxf")
                        nc.vector.tensor_tensor(
                            out=gidxf,
                            in0=locf.rearrange("p g s -> p (g s)"),
                            in1=goffs_f.rearrange("p g s -> p (g s)"),
                            op=AL.add,
                        )
                        cand = v8.rearrange("p g s -> p (g s)")
                        t8a = small.tile([P, 8], F32, tag="t8a")
                        t8b = small.tile([P, 8], F32, tag="t8b")
                        p16 = small.tile([P, K], U16, tag="p16")
                        cand2 = small.tile([P, P], F32, tag="cand2")
                        nc.vector.max(out=t8a, in_=cand)
                        nc.vector.max_index(out=p16[:, 0:8], in_max=t8a, in_values=cand)
                        nc.vector.match_replace(
                            out=cand2, in_to_replace=t8a, in_values=cand,
                            imm_value=NEG_BIG,
                        )
                        nc.vector.max(out=t8b, in_=cand2)
                        nc.vector.max_index(
                            out=p16[:, 8:16], in_max=t8b, in_values=cand2
                        )
                        p16f = small.tile([P, K], F32, tag="p16f")
                        nc.vector.tensor_copy(p16f, p16)

                        gself = small.tile([P, K], F32, tag="gself")
                        sttscr = small.tile([P, P], F32, tag="sttscr")
                        for j in range(K):
                            nc.vector.scalar_tensor_tensor(
                                out=sttscr,
                                in0=iota_f,
                                scalar=p16f[:, j : j + 1],
                                in1=gidxf,
                                op0=AL.is_equal,
                                op1=AL.mult,
                                accum_out=gself[:, j : j + 1],
                            )
                        gt_ps = gtpsum.tile([16, P], F32, tag="gt_ps")
                        nc.tensor.transpose(gt_ps, gself, identity)
                        gtw = small.tile([16, P], I16, tag="gtw")
                        nc.vector.tensor_copy(gtw, gt_ps)
                        nc.sync.dma_start(
                            out=widx_d[
                                :, w * rr : (w + 1) * rr,
                                SOFF[ci] : SOFF[ci] + sc,
                            ],
                            in_=gtw.rearrange("j (r s) -> j r s", r=rr),
                        )

                        if IDX_OUT:
                            idxu = small.tile([P, K], U32, tag="idxu")
                            nc.vector.tensor_copy(idxu, gself)
                            nc.sync.dma_start(
                                out=idx_d[ts * P : (ts + 1) * P], in_=idxu
                            )

                        for (src, j) in ISSUE_AT.get((ci, w), []):
                            gather_issue(src, j)
                        for (src, j) in FOLD_AT.get((ci, w), []):
                            gather_max(src, j)
                    if ci in END_OPS:
                        src, js = END_OPS[ci]
                        for j in js:
                            gather_max(src, j)
                        gather_finish(src)
                lastc = NCH - 1
                gather_begin(lastc)
                for j in range(K):
                    gather_issue(lastc, j)
                    gather_max(lastc, j - 1, force_dve=True)
                gather_max(lastc, K - 1, force_dve=True)
                gather_finish(lastc)

    nc.compile()
    return nc


_NC_CACHE = None


def _get_nc():
    global _NC_CACHE
    if _NC_CACHE is None:
        _NC_CACHE = build_nc()
    return _NC_CACHE


def _row_perm():
    """perm[ts*128 + p] = own-half row index handled by tile ts, partition p.

    gather order i = ch + 16*s with ch = w*rr + p//SC, s = p % SC
    (rr = 16/CT, SC = 8*CT) -> row n = off + i."""
    perm = np.empty(NR, dtype=np.int64)
    for ci in range(NCH):
        ct = CHUNKS[ci]
        rr = 16 // ct
        sc = 8 * ct
        off = 128 * COFF[ci]
        for w in range(ct):
            ts = COFF[ci] + w
            p = np.arange(P)
            perm[ts * P + p] = off + (w * rr + p // sc) + 16 * (p % sc)
    return perm


_PERM = _row_perm()


def _shard_inputs(q, q_pos, W, b):
    q = np.ascontiguousarray(np.asarray(q, dtype=np.float32))
    q_pos = np.ascontiguousarray(np.asarray(q_pos, dtype=np.float32))
    W = np.ascontiguousarray(np.asarray(W, dtype=np.float32))
    b = np.ascontiguousarray(np.asarray(b, dtype=np.float32)).reshape(1, C)
    in_maps = []
    for core in range(8):
        bi, h = divmod(core, 2)
        rows = slice(h * NR, (h + 1) * NR)
        posb = q_pos[bi]                          # [8192, 3] global order
        posr = np.ascontiguousarray(posb[rows])   # own rows
        pn2 = (posr * posr).sum(-1, dtype=np.float32)
        pm2 = (posb * posb).sum(-1, dtype=np.float32)
        ls2_rows = (posr * np.float32(2.0)).T     # [3, NR] own-row order
        ls2 = np.ascontiguousarray(ls2_rows[:, _PERM])
        pn2s = np.ascontiguousarray((-pn2)[_PERM].reshape(NT, P).T)
        in_maps.append(
            {
                "qT": np.ascontiguousarray(q[bi].T),
                "qrT": np.ascontiguousarray(q[bi, rows].T),
                "ls2": ls2,
                "rs": np.ascontiguousarray(posb.T),
                "pn2s": pn2s,
                "pm2n": np.ascontiguousarray((-pm2).reshape(1, N)),
                "w": W,
                "bvec": b,
            }
        )
    return in_maps


def run_on_hw(q, q_pos, W, b, trace=False):
    """Run the SPMD kernel on the 8 cores; returns (out[4,8192,256], results)."""
    nc = _get_nc()
    in_maps = _shard_inputs(q, q_pos, W, b)
    res = run_bass_kernel_spmd(nc, in_maps, core_ids=list(range(8)), trace=trace)
    out = np.empty((4, N, C), dtype=np.float32)
    for core in range(8):
        bi, h = divmod(core, 2)
        out[bi, h * NR : (h + 1) * NR] = res.results[core]["out"]
    return out, res


def kernel(q, q_pos, W, b, k):
    assert int(k) == K, f"kernel hardcodes k=16, got {k}"
    out, _ = run_on_hw(q, q_pos, W, b)
    return out
